# revision 39
# baseline (speedup 1.0000x reference)
"""Trainium2 Bass kernel for the token-scan problem.

Math: the reference scans T=128 tokens updating (x, rho) and emits
concat([x_T, y_T, v*_T, rho_T.ravel()]).  The x-recurrence depends only on
the (known) token sequence, so the scan unrolls into dense matmuls:

  V    = token_emb[tokens]                 [T, d]
  R    = relu(Dx @ V^T)                    [n, T]
  x_f  = R @ ones  (row sums)
  h    = R^T x_f                           [T]
  a    = vwu^T h  (vwu = U @ (V*w), U = triu-ones; w = decay weights)
  y    = relu(Dy @ ln(a)) * x_f            [n]
  v*   = ln(E @ y)                         [d]
  rho  = vwp^T R^T (vwp = U @ (V*w'))      [d, n]

Sharding: n split across 8 cores (Dx/Dy rows, E/rho columns, x/y slices).
Cross-core comm: ONE AllReduce of the centered a-partial, shipped in
column layout [128, 2] (the centering  a - mean(a)*ones  is folded into
vwu on the host: vwu'' = (vwu - rowmean) * 2^-10, so the device-side
partial comes out of the PE pre-centered and pre-scaled, in columns --
no transpose matmuls and no mean matmul needed).  The final E@y partial
sums ([d] per core) are reduced + layernormed on the host during unshard.

Schedule (cost-model driven): loads go on the SP and Pool rings only (the
Act ring opens with its 1283ns activation-table load, so it gets no
loads); rho is computed chunk-by-chunk and written out in 8 [128,512]
granules spread over all three rings, ordered so the last granule's
copy+DMA chain starts as early as possible.  Everything -- including the
rho writeback -- completes before the AllReduce; the tail after it is
just: a_out fetch, y/v* chain, one combined y|vs output DMA.

Precision: big operands ship as bf16 (Dx, rho out) and fp8-e4m3 (Dy x16,
E x64; these only feed the small y/v* output segments; ln is scale
invariant so only Dy's scale needs undoing, folded into the relu).
All matmuls accumulate in fp32 PSUM.
"""

import numpy as np
import ml_dtypes

N, D, V_VOCAB, T = 16384, 256, 32000, 128
DECAY = 0.97
N_CORES = 8
NS = N // N_CORES           # 2048 rows per core
NT = NS // 128              # 16 tiles of 128
YSCL = 2.0 ** -5            # y -> fp8 scale (ln(E@y) is scale invariant)

_cache = {}

# chunk processing order == load-completion order (c3 on Pool lands first,
# then c0/SP, c2/Pool, c1/SP)
CHUNK_ORDER = [3, 0, 2, 1]


def _build():
    import concourse.bacc as bacc
    import concourse.mybir as mybir
    import concourse.tile as tile

    f32 = mybir.dt.float32
    bf16 = mybir.dt.bfloat16
    f8 = mybir.dt.float8e4
    AF = mybir.ActivationFunctionType
    ALU = mybir.AluOpType

    nc = bacc.Bacc("TRN2", target_bir_lowering=False, debug=False,
                   num_devices=N_CORES)

    i_dxts = nc.dram_tensor("dxts", [128, 2 * NS], bf16, kind="ExternalInput")
    i_dyts = nc.dram_tensor("dyts", [128, 2 * NS], bf16, kind="ExternalInput")
    i_ets = nc.dram_tensor("ets", [128, NT * 256], bf16, kind="ExternalInput")
    i_consts = nc.dram_tensor("consts", [128, 770], bf16, kind="ExternalInput")

    o_x = nc.dram_tensor("out_x", [128, NT], bf16, kind="ExternalOutput")
    o_a = nc.dram_tensor("out_a", [128, 2], bf16, kind="ExternalOutput")
    # combined tail output: cols 0..NT-1 = y (bf16), cols NT..NT+1 = vs
    o_yv = nc.dram_tensor("out_yv", [128, NT + 2], bf16, kind="ExternalOutput")
    o_rho = nc.dram_tensor("out_rho", [256, NS], bf16, kind="ExternalOutput")

    with tile.TileContext(nc) as tc:
        with (
            tc.tile_pool(name="persist", bufs=1) as pp,
            tc.tile_pool(name="rhobuf", bufs=2) as wp,
            tc.tile_pool(name="psBig", bufs=4, space="PSUM") as psBig,
            tc.tile_pool(name="psRc", bufs=2, space="PSUM") as psRc,
            tc.tile_pool(name="psT", bufs=1, space="PSUM") as psT,
            tc.tile_pool(name="dram", bufs=1, space="DRAM") as dram,
        ):
            consts = pp.tile([128, 770], bf16)
            dxts = pp.tile([128, 2 * NS], bf16)
            dyts = pp.tile([128, 2 * NS], bf16)
            ets = pp.tile([128, NT * 256], bf16)

            def ccols(c):
                return slice(c * 1024, (c + 1) * 1024)

            # SP ring: consts, dxts chunks 0, 1
            nc.sync.dma_start(consts[:], i_consts[:])
            nc.sync.dma_start(dxts[:, ccols(0)], i_dxts[:, ccols(0)])
            nc.sync.dma_start(dxts[:, ccols(1)], i_dxts[:, ccols(1)])
            # Pool ring: dxts chunk 3, chunk 2 (kept short so the Pool-ring
            # rho writes get early exec slots); tail-only operands on SP
            nc.gpsimd.dma_start(dxts[:, ccols(3)], i_dxts[:, ccols(3)])
            nc.gpsimd.dma_start(dxts[:, ccols(2)], i_dxts[:, ccols(2)])
            nc.sync.dma_start(dyts[:], i_dyts[:])
            nc.sync.dma_start(ets[:], i_ets[:])

            vts = consts[:, 0:256]
            vwu = consts[:, 256:512]   # pre-centered, pre-scaled (2^-10)
            vwp = consts[:, 512:768]
            onec = consts[:, 768:769]  # column of ones (for x_f row sums)

            # one PSUM bank for all small tiles:
            # cols 0-15 y, 16-17 a-cols, 18-19 vs, 20 h
            tail_ps = psT.tile([128, 40], f32, tag="tail")

            rcols = pp.tile([128, NT * 128], bf16)
            rt = pp.tile([128, NS], bf16)
            xfb = pp.tile([128, NT], bf16)
            rho_sbs = []
            for dc in range(2):
                rho_sb = wp.tile([128, NS], bf16, tag="rho")
                rho_sbs.append(rho_sb)

            def rc_chunk(c, eng):
                rc_ps = psRc.tile([128, 512], f32, tag="rc")
                for j in range(4):
                    base = c * 1024 + j * 128
                    for h in range(2):
                        nc.tensor.matmul(
                            rc_ps[:, j * 128:(j + 1) * 128],
                            lhsT=dxts[:, base + h * 512: base + h * 512 + 128],
                            rhs=vts[:, h * 128:(h + 1) * 128],
                            start=(h == 0), stop=(h == 1))
                dst = rcols[:, c * 512:(c + 1) * 512]
                if eng == 0:
                    nc.vector.tensor_scalar_max(dst, rc_ps[:], 0.0)
                else:
                    nc.scalar.activation(dst, rc_ps[:], AF.Relu)

            def rt_chunk(c, e):
                rt_ps = psBig.tile([128, 512], f32, tag="big")
                for h in range(2):
                    nc.tensor.matmul(
                        rt_ps[:],
                        lhsT=vts[:, h * 128:(h + 1) * 128],
                        rhs=dxts[:, c * 1024 + h * 512: c * 1024 + h * 512 + 512],
                        start=(h == 0), stop=(h == 1))
                dst = rt[:, c * 512:(c + 1) * 512]
                if e == 0:
                    nc.vector.tensor_scalar_max(dst, rt_ps[:], 0.0)
                elif e == 1:
                    nc.scalar.activation(dst, rt_ps[:], AF.Relu)
                else:
                    nc.vector.tensor_scalar_max(
                        rt[:, c * 512:c * 512 + 256], rt_ps[:, 0:256], 0.0)
                    nc.scalar.activation(
                        rt[:, c * 512 + 256:(c + 1) * 512],
                        rt_ps[:, 256:512], AF.Relu)
                # x_f tile sums: xf[n] = sum_t rt[t, n] per 128-tile
                for j in range(4):
                    i = c * 4 + j
                    nc.tensor.matmul(
                        tail_ps[:, 24 + i:25 + i],
                        lhsT=rt[:, c * 512 + j * 128: c * 512 + (j + 1) * 128],
                        rhs=onec, start=True, stop=True)

            def rho_chunk(c, copy_eng, rings=None):
                for dc in range(2):
                    rho_ps = psBig.tile([128, 512], f32, tag="big")
                    nc.tensor.matmul(rho_ps[:],
                                     lhsT=vwp[:, dc * 128:(dc + 1) * 128],
                                     rhs=rt[:, c * 512:(c + 1) * 512],
                                     start=True, stop=True)
                    dst = rho_sbs[dc][:, c * 512:(c + 1) * 512]
                    if copy_eng[dc] == 0:
                        nc.vector.tensor_copy(dst, rho_ps[:])
                    else:
                        nc.scalar.activation(dst, rho_ps[:], AF.Copy)
                    if rings is not None:
                        rings[dc].dma_start(
                            o_rho[dc * 128:(dc + 1) * 128,
                                  c * 512:(c + 1) * 512], dst)

            # ---- PE program ----
            c0, c1, c2, c3 = CHUNK_ORDER
            rc_chunk(c0, 0)
            rt_chunk(c0, 1)
            rc_chunk(c1, 0)
            rt_chunk(c1, 1)
            rho_chunk(c0, [0, 1], rings=[nc.gpsimd, nc.gpsimd])
            rc_chunk(c2, 0)
            rt_chunk(c2, 1)
            rho_chunk(c1, [0, 1], rings=[nc.gpsimd, nc.gpsimd])
            rc_chunk(c3, 1)
            rt_chunk(c3, 2)

            # ---- a-chain: h = R^T x_f ; a_cols = vwu''^T h ----
            nc.vector.tensor_copy(xfb[:], tail_ps[:, 24:40])
            for i in range(NT):
                nc.tensor.matmul(tail_ps[:, 20:21],
                                 lhsT=rcols[:, i * 128:(i + 1) * 128],
                                 rhs=xfb[:, i:i + 1],
                                 start=(i == 0), stop=(i == NT - 1))
            h_sb = pp.tile([128, 1], bf16)
            nc.vector.tensor_copy(h_sb[:], tail_ps[:, 20:21])
            for dc in range(2):
                nc.tensor.matmul(tail_ps[:, 16 + dc:17 + dc],
                                 lhsT=vwu[:, dc * 128:(dc + 1) * 128],
                                 rhs=h_sb[:], start=True, stop=True)
            a_sb = pp.tile([128, 2], bf16)
            nc.vector.tensor_copy(a_sb[:], tail_ps[:, 16:18])

            a_in = dram.tile([128, 2], bf16)
            a_out = dram.tile([128, 2], bf16)
            nc.sync.dma_start(a_in[:], a_sb[:])
            nc.sync.dma_start(o_x[:], xfb[:])

            rho_chunk(c2, [0, 1], rings=[nc.scalar, nc.gpsimd])
            rho_ps3 = psBig.tile([128, 512], f32, tag="big")
            nc.tensor.matmul(rho_ps3[:], lhsT=vwp[:, 0:128],
                             rhs=rt[:, c3 * 512:(c3 + 1) * 512],
                             start=True, stop=True)
            dst30 = rho_sbs[0][:, c3 * 512:(c3 + 1) * 512]
            nc.vector.tensor_copy(dst30, rho_ps3[:])
            nc.sync.dma_start(o_rho[0:128, c3 * 512:(c3 + 1) * 512], dst30)
            rho_ps3b = psBig.tile([128, 512], f32, tag="big")
            nc.tensor.matmul(rho_ps3b[:], lhsT=vwp[:, 128:256],
                             rhs=rt[:, c3 * 512:(c3 + 1) * 512],
                             start=True, stop=True)

            nc.gpsimd.collective_compute(
                "AllReduce", ALU.add,
                replica_groups=[list(range(N_CORES))],
                ins=[a_in.opt()], outs=[a_out.opt()],
            )

            # ---- tail ----
            afull = pp.tile([128, 2], bf16)
            nc.sync.dma_start(afull[:], a_out[:])
            nc.gpsimd.dma_start(o_a[:], afull[:])

            # y = relu(Dy @ a_cols)*2^-10 * x_f  (Dy bf16, a read directly)
            for i in range(NT):
                for h in range(2):
                    nc.tensor.matmul(
                        tail_ps[:, i:i + 1],
                        lhsT=dyts[:, h * NS + i * 128: h * NS + (i + 1) * 128],
                        rhs=afull[:, h:h + 1],
                        start=(h == 0), stop=(h == 1))
            yv_sb = pp.tile([128, NT + 2], bf16)
            nc.vector.scalar_tensor_tensor(
                yv_sb[:, 0:NT], tail_ps[:, 0:NT], 0.0, xfb[:],
                op0=ALU.max, op1=ALU.mult)
            nc.sync.dma_start(o_yv[:, 0:NT], yv_sb[:, 0:NT])
            # deferred last rho granule: copy gated behind the afull-dependent
            # stt on DVE, write lands inside the collective shadow
            dst31 = rho_sbs[1][:, c3 * 512:(c3 + 1) * 512]
            nc.vector.tensor_copy(dst31, rho_ps3b[:])
            nc.scalar.dma_start(o_rho[128:256, c3 * 512:(c3 + 1) * 512], dst31)

            # vs partial = E @ y  (E bf16; host ln is scale-invariant)
            for h in range(2):
                for i in range(NT):
                    nc.tensor.matmul(
                        tail_ps[:, 18 + h:19 + h],
                        lhsT=ets[:, i * 256 + h * 128: i * 256 + (h + 1) * 128],
                        rhs=yv_sb[:, i:i + 1],
                        start=(i == 0), stop=(i == NT - 1))
            nc.scalar.activation(yv_sb[:, NT:NT + 2], tail_ps[:, 18:20],
                                 AF.Copy)
            nc.scalar.dma_start(o_yv[:, NT:NT + 2], yv_sb[:, NT:NT + 2])

    nc.finalize()
    return nc


def _host_prep(E, Dx, Dy, token_emb, tokens):
    bf = ml_dtypes.bfloat16
    f8 = ml_dtypes.float8_e4m3fn
    E = np.asarray(E, dtype=np.float32)
    Dx = np.asarray(Dx, dtype=np.float32)
    Dy = np.asarray(Dy, dtype=np.float32)
    token_emb = np.asarray(token_emb, dtype=np.float32)
    tokens = np.asarray(tokens).astype(np.int64)

    v = np.ascontiguousarray(token_emb[tokens])          # [T, d]
    vts = np.concatenate([v[:, :128].T, v[:, 128:].T], axis=1)  # [128, 256]
    j = np.arange(T)
    w = (DECAY ** ((T - 1) - j)).astype(np.float32)
    w[T - 1] = 0.0
    wp = (DECAY ** (T - j)).astype(np.float32)
    u = np.triu(np.ones((T, T), dtype=np.float32))
    vwu = u @ (v * w[:, None])                           # [T, d]
    vwp = u @ (v * wp[:, None])
    # fold the ln centering and the 2^-10 pre-scale into vwu:
    # a_cols = vwu''^T h  comes out centered+scaled on the device.
    vwu = (vwu - vwu.mean(axis=1, keepdims=True)) * 2.0 ** -10
    onec = np.ones((128, 1), np.float32)
    pad = np.zeros((128, 1), np.float32)
    consts = np.ascontiguousarray(
        np.concatenate([vts, vwu, vwp, onec, pad], axis=1)).astype(bf)

    in_maps = []
    for k in range(N_CORES):
        sl = slice(k * NS, (k + 1) * NS)
        dx_s = Dx[sl]                                    # [NS, 256]
        dy_s = Dy[sl]
        e_s = E[:, sl]                                   # [256, NS]
        dxts = np.empty((128, 2 * NS), np.float32)
        for c in range(4):
            nsl = slice(c * 512, (c + 1) * 512)
            dxts[:, c * 1024: c * 1024 + 512] = dx_s[nsl, :128].T
            dxts[:, c * 1024 + 512: (c + 1) * 1024] = dx_s[nsl, 128:].T
        dyts = np.concatenate([dy_s[:, :128].T, dy_s[:, 128:].T],
                              axis=1)
        ets = np.concatenate(
            [e_s[:, i * 128:(i + 1) * 128].T for i in range(NT)],
            axis=1)
        in_maps.append({
            "dxts": np.ascontiguousarray(dxts).astype(bf),
            "dyts": np.ascontiguousarray(dyts).astype(bf),
            "ets": np.ascontiguousarray(ets).astype(bf),
            "consts": consts,
        })
    return in_maps


def _ln_host(z, eps=1e-6):
    m = z.mean()
    s = z.std(ddof=1)
    return (z - m) / (s + eps)


def kernel(E, Dx, Dy, token_emb, tokens, _trace=False):
    from concourse.bass_utils import run_bass_kernel_spmd

    key = "nc"
    if key not in _cache:
        _cache[key] = _build()
    nc = _cache[key]

    in_maps = _host_prep(E, Dx, Dy, token_emb, tokens)
    res = run_bass_kernel_spmd(nc, in_maps, core_ids=list(range(N_CORES)),
                               trace=_trace)
    _cache["last_result"] = res

    r = res.results
    x_full = np.concatenate(
        [np.asarray(r[k]["out_x"], np.float32).T.ravel()
         for k in range(N_CORES)])
    # out_a holds (a - mean) * 2^-10 in column layout [128, 2]
    a_cent = np.asarray(r[0]["out_a"], np.float32).T.ravel() * 2.0 ** 10
    yfac = 1024.0 / (a_cent.std(ddof=1) + 1e-6)
    y_full = np.concatenate(
        [np.asarray(r[k]["out_yv"]).astype(np.float32)[:, 0:NT].T.ravel()
         * yfac for k in range(N_CORES)])
    vs_raw = np.zeros(256, np.float64)
    for k in range(N_CORES):
        vs_raw += np.asarray(r[k]["out_yv"]).astype(np.float32)[:, NT:].T.ravel()
    vs = _ln_host(vs_raw.astype(np.float32))
    rho = np.concatenate(
        [np.asarray(r[k]["out_rho"]).astype(np.float32)
         for k in range(N_CORES)], axis=1)
    return np.concatenate([x_full, y_full, vs, rho.ravel()]).astype(np.float32)


# revision 40
# speedup vs baseline: 1.0056x; 1.0056x over previous
"""Trainium2 Bass kernel for the token-scan problem.

Math: the reference scans T=128 tokens updating (x, rho) and emits
concat([x_T, y_T, v*_T, rho_T.ravel()]).  The x-recurrence depends only on
the (known) token sequence, so the scan unrolls into dense matmuls:

  V    = token_emb[tokens]                 [T, d]
  R    = relu(Dx @ V^T)                    [n, T]
  x_f  = R @ ones  (row sums)
  h    = R^T x_f                           [T]
  a    = vwu^T h  (vwu = U @ (V*w), U = triu-ones; w = decay weights)
  y    = relu(Dy @ ln(a)) * x_f            [n]
  v*   = ln(E @ y)                         [d]
  rho  = vwp^T R^T (vwp = U @ (V*w'))      [d, n]

Sharding: n split across 8 cores (Dx/Dy rows, E/rho columns, x/y slices).
Cross-core comm: ONE AllReduce of the centered a-partial, shipped in
column layout [128, 2] (the centering  a - mean(a)*ones  is folded into
vwu on the host: vwu'' = (vwu - rowmean) * 2^-10, so the device-side
partial comes out of the PE pre-centered and pre-scaled, in columns --
no transpose matmuls and no mean matmul needed).  The final E@y partial
sums ([d] per core) are reduced + layernormed on the host during unshard.

Schedule (cost-model driven): loads go on the SP and Pool rings only (the
Act ring opens with its 1283ns activation-table load, so it gets no
loads); rho is computed chunk-by-chunk and written out in 8 [128,512]
granules spread over all three rings, ordered so the last granule's
copy+DMA chain starts as early as possible.  Everything -- including the
rho writeback -- completes before the AllReduce; the tail after it is
just: a_out fetch, y/v* chain, one combined y|vs output DMA.

Precision: big operands ship as bf16 (Dx, rho out) and fp8-e4m3 (Dy x16,
E x64; these only feed the small y/v* output segments; ln is scale
invariant so only Dy's scale needs undoing, folded into the relu).
All matmuls accumulate in fp32 PSUM.
"""

import numpy as np
import ml_dtypes

N, D, V_VOCAB, T = 16384, 256, 32000, 128
DECAY = 0.97
N_CORES = 8
NS = N // N_CORES           # 2048 rows per core
NT = NS // 128              # 16 tiles of 128
YSCL = 2.0 ** -5            # y -> fp8 scale (ln(E@y) is scale invariant)

_cache = {}

# chunk processing order == load-completion order (c3 on Pool lands first,
# then c0/SP, c2/Pool, c1/SP)
CHUNK_ORDER = [3, 0, 2, 1]


def _build():
    import concourse.bacc as bacc
    import concourse.mybir as mybir
    import concourse.tile as tile

    f32 = mybir.dt.float32
    bf16 = mybir.dt.bfloat16
    f8 = mybir.dt.float8e4
    AF = mybir.ActivationFunctionType
    ALU = mybir.AluOpType

    nc = bacc.Bacc("TRN2", target_bir_lowering=False, debug=False,
                   num_devices=N_CORES)

    i_dxts = nc.dram_tensor("dxts", [128, 2 * NS], bf16, kind="ExternalInput")
    i_dyts = nc.dram_tensor("dyts", [128, 2 * NS], bf16, kind="ExternalInput")
    i_ets = nc.dram_tensor("ets", [128, NT * 256], bf16, kind="ExternalInput")
    i_consts = nc.dram_tensor("consts", [128, 770], bf16, kind="ExternalInput")

    o_x = nc.dram_tensor("out_x", [128, NT], bf16, kind="ExternalOutput")
    o_a = nc.dram_tensor("out_a", [128, 2], bf16, kind="ExternalOutput")
    # combined tail output: cols 0..NT-1 = y (bf16), cols NT..NT+1 = vs
    o_yv = nc.dram_tensor("out_yv", [128, NT + 2], bf16, kind="ExternalOutput")
    o_rho = nc.dram_tensor("out_rho", [256, NS], bf16, kind="ExternalOutput")

    with tile.TileContext(nc) as tc:
        with (
            tc.tile_pool(name="persist", bufs=1) as pp,
            tc.tile_pool(name="rhobuf", bufs=2) as wp,
            tc.tile_pool(name="psBig", bufs=4, space="PSUM") as psBig,
            tc.tile_pool(name="psRc", bufs=2, space="PSUM") as psRc,
            tc.tile_pool(name="psT", bufs=1, space="PSUM") as psT,
            tc.tile_pool(name="dram", bufs=1, space="DRAM") as dram,
        ):
            consts = pp.tile([128, 770], bf16)
            dxts = pp.tile([128, 2 * NS], bf16)
            dyts = pp.tile([128, 2 * NS], bf16)
            ets = pp.tile([128, NT * 256], bf16)

            def ccols(c):
                return slice(c * 1024, (c + 1) * 1024)

            # SP ring: consts, dxts chunks 0, 1
            nc.sync.dma_start(consts[:], i_consts[:])
            nc.sync.dma_start(dxts[:, ccols(0)], i_dxts[:, ccols(0)])
            nc.sync.dma_start(dxts[:, ccols(1)], i_dxts[:, ccols(1)])
            # Pool ring: dxts chunk 3, chunk 2 (kept short so the Pool-ring
            # rho writes get early exec slots); tail-only operands on SP
            nc.gpsimd.dma_start(dxts[:, ccols(3)], i_dxts[:, ccols(3)])
            nc.gpsimd.dma_start(dxts[:, ccols(2)], i_dxts[:, ccols(2)])
            nc.sync.dma_start(dyts[:], i_dyts[:])
            nc.sync.dma_start(ets[:], i_ets[:])

            vts = consts[:, 0:256]
            vwu = consts[:, 256:512]   # pre-centered, pre-scaled (2^-10)
            vwp = consts[:, 512:768]
            onec = consts[:, 768:769]  # column of ones (for x_f row sums)

            # one PSUM bank for all small tiles:
            # cols 0-15 y, 16-17 a-cols, 18-19 vs, 20 h
            tail_ps = psT.tile([128, 40], f32, tag="tail")

            rcols = pp.tile([128, NT * 128], bf16)
            rt = pp.tile([128, NS], bf16)
            xfb = pp.tile([128, NT], bf16)
            rho_sbs = []
            for dc in range(2):
                rho_sb = wp.tile([128, NS], bf16, tag="rho")
                rho_sbs.append(rho_sb)

            def rc_chunk(c, eng):
                rc_ps = psRc.tile([128, 512], f32, tag="rc")
                for j in range(4):
                    base = c * 1024 + j * 128
                    for h in range(2):
                        nc.tensor.matmul(
                            rc_ps[:, j * 128:(j + 1) * 128],
                            lhsT=dxts[:, base + h * 512: base + h * 512 + 128],
                            rhs=vts[:, h * 128:(h + 1) * 128],
                            start=(h == 0), stop=(h == 1))
                dst = rcols[:, c * 512:(c + 1) * 512]
                if eng == 0:
                    nc.vector.tensor_scalar_max(dst, rc_ps[:], 0.0)
                else:
                    nc.scalar.activation(dst, rc_ps[:], AF.Relu)

            def rt_chunk(c, e):
                rt_ps = psBig.tile([128, 512], f32, tag="big")
                for h in range(2):
                    nc.tensor.matmul(
                        rt_ps[:],
                        lhsT=vts[:, h * 128:(h + 1) * 128],
                        rhs=dxts[:, c * 1024 + h * 512: c * 1024 + h * 512 + 512],
                        start=(h == 0), stop=(h == 1))
                dst = rt[:, c * 512:(c + 1) * 512]
                if e == 0:
                    nc.vector.tensor_scalar_max(dst, rt_ps[:], 0.0)
                elif e == 1:
                    nc.scalar.activation(dst, rt_ps[:], AF.Relu)
                else:
                    nc.vector.tensor_scalar_max(
                        rt[:, c * 512:c * 512 + 256], rt_ps[:, 0:256], 0.0)
                    nc.scalar.activation(
                        rt[:, c * 512 + 256:(c + 1) * 512],
                        rt_ps[:, 256:512], AF.Relu)
                # x_f tile sums: xf[n] = sum_t rt[t, n] per 128-tile
                for j in range(4):
                    i = c * 4 + j
                    nc.tensor.matmul(
                        tail_ps[:, 24 + i:25 + i],
                        lhsT=rt[:, c * 512 + j * 128: c * 512 + (j + 1) * 128],
                        rhs=onec, start=True, stop=True)

            def rho_chunk(c, copy_eng, rings=None):
                for dc in range(2):
                    rho_ps = psBig.tile([128, 512], f32, tag="big")
                    nc.tensor.matmul(rho_ps[:],
                                     lhsT=vwp[:, dc * 128:(dc + 1) * 128],
                                     rhs=rt[:, c * 512:(c + 1) * 512],
                                     start=True, stop=True)
                    dst = rho_sbs[dc][:, c * 512:(c + 1) * 512]
                    if copy_eng[dc] == 0:
                        nc.vector.tensor_copy(dst, rho_ps[:])
                    else:
                        nc.scalar.activation(dst, rho_ps[:], AF.Copy)
                    if rings is not None:
                        rings[dc].dma_start(
                            o_rho[dc * 128:(dc + 1) * 128,
                                  c * 512:(c + 1) * 512], dst)

            # ---- PE program ----
            c0, c1, c2, c3 = CHUNK_ORDER
            rc_chunk(c0, 0)
            rt_chunk(c0, 1)
            rc_chunk(c1, 0)
            rt_chunk(c1, 1)
            rho_chunk(c0, [0, 1], rings=[nc.gpsimd, nc.gpsimd])
            rc_chunk(c2, 0)
            rt_chunk(c2, 1)
            rho_chunk(c1, [0, 1], rings=[nc.gpsimd, nc.gpsimd])
            rc_chunk(c3, 1)
            rt_chunk(c3, 2)

            # ---- a-chain: h = R^T x_f ; a_cols = vwu''^T h ----
            nc.vector.tensor_copy(xfb[:], tail_ps[:, 24:40])
            for i in range(NT):
                nc.tensor.matmul(tail_ps[:, 20:21],
                                 lhsT=rcols[:, i * 128:(i + 1) * 128],
                                 rhs=xfb[:, i:i + 1],
                                 start=(i == 0), stop=(i == NT - 1))
            h_sb = pp.tile([128, 1], bf16)
            nc.vector.tensor_copy(h_sb[:], tail_ps[:, 20:21])
            for dc in range(2):
                nc.tensor.matmul(tail_ps[:, 16 + dc:17 + dc],
                                 lhsT=vwu[:, dc * 128:(dc + 1) * 128],
                                 rhs=h_sb[:], start=True, stop=True)
            a_sb = pp.tile([128, 2], bf16)
            nc.vector.tensor_copy(a_sb[:], tail_ps[:, 16:18])

            a_in = dram.tile([128, 2], bf16)
            a_out = dram.tile([128, 2], bf16)
            nc.sync.dma_start(a_in[:], a_sb[:])
            nc.sync.dma_start(o_x[:], xfb[:])

            rho_chunk(c2, [0, 1], rings=[nc.scalar, nc.gpsimd])
            rho_chunk(c3, [0, 1], rings=[nc.sync, nc.scalar])

            nc.gpsimd.collective_compute(
                "AllReduce", ALU.add,
                replica_groups=[list(range(N_CORES))],
                ins=[a_in.opt()], outs=[a_out.opt()],
            )

            # ---- tail ----
            afull = pp.tile([128, 2], bf16)
            nc.sync.dma_start(afull[:], a_out[:])
            nc.gpsimd.dma_start(o_a[:], afull[:])

            # y = relu(Dy @ a_cols)*2^-10 * x_f  (Dy bf16, a read directly)
            for i in range(NT):
                for h in range(2):
                    nc.tensor.matmul(
                        tail_ps[:, i:i + 1],
                        lhsT=dyts[:, h * NS + i * 128: h * NS + (i + 1) * 128],
                        rhs=afull[:, h:h + 1],
                        start=(h == 0), stop=(h == 1))
            yv_sb = pp.tile([128, NT + 2], bf16)
            nc.vector.scalar_tensor_tensor(
                yv_sb[:, 0:NT], tail_ps[:, 0:NT], 0.0, xfb[:],
                op0=ALU.max, op1=ALU.mult)
            nc.sync.dma_start(o_yv[:, 0:NT], yv_sb[:, 0:NT])

            # vs partial = E @ y  (E bf16; host ln is scale-invariant)
            for h in range(2):
                for i in range(NT):
                    nc.tensor.matmul(
                        tail_ps[:, 18 + h:19 + h],
                        lhsT=ets[:, i * 256 + h * 128: i * 256 + (h + 1) * 128],
                        rhs=yv_sb[:, i:i + 1],
                        start=(i == 0), stop=(i == NT - 1))
            nc.scalar.activation(yv_sb[:, NT:NT + 2], tail_ps[:, 18:20],
                                 AF.Copy)
            nc.scalar.dma_start(o_yv[:, NT:NT + 2], yv_sb[:, NT:NT + 2])

    nc.finalize()
    return nc


def _host_prep(E, Dx, Dy, token_emb, tokens):
    bf = ml_dtypes.bfloat16
    f8 = ml_dtypes.float8_e4m3fn
    E = np.asarray(E, dtype=np.float32)
    Dx = np.asarray(Dx, dtype=np.float32)
    Dy = np.asarray(Dy, dtype=np.float32)
    token_emb = np.asarray(token_emb, dtype=np.float32)
    tokens = np.asarray(tokens).astype(np.int64)

    v = np.ascontiguousarray(token_emb[tokens])          # [T, d]
    vts = np.concatenate([v[:, :128].T, v[:, 128:].T], axis=1)  # [128, 256]
    j = np.arange(T)
    w = (DECAY ** ((T - 1) - j)).astype(np.float32)
    w[T - 1] = 0.0
    wp = (DECAY ** (T - j)).astype(np.float32)
    u = np.triu(np.ones((T, T), dtype=np.float32))
    vwu = u @ (v * w[:, None])                           # [T, d]
    vwp = u @ (v * wp[:, None])
    # fold the ln centering and the 2^-10 pre-scale into vwu:
    # a_cols = vwu''^T h  comes out centered+scaled on the device.
    vwu = (vwu - vwu.mean(axis=1, keepdims=True)) * 2.0 ** -10
    onec = np.ones((128, 1), np.float32)
    pad = np.zeros((128, 1), np.float32)
    consts = np.ascontiguousarray(
        np.concatenate([vts, vwu, vwp, onec, pad], axis=1)).astype(bf)

    in_maps = []
    for k in range(N_CORES):
        sl = slice(k * NS, (k + 1) * NS)
        dx_s = Dx[sl]                                    # [NS, 256]
        dy_s = Dy[sl]
        e_s = E[:, sl]                                   # [256, NS]
        dxts = np.empty((128, 2 * NS), np.float32)
        for c in range(4):
            nsl = slice(c * 512, (c + 1) * 512)
            dxts[:, c * 1024: c * 1024 + 512] = dx_s[nsl, :128].T
            dxts[:, c * 1024 + 512: (c + 1) * 1024] = dx_s[nsl, 128:].T
        dyts = np.concatenate([dy_s[:, :128].T, dy_s[:, 128:].T],
                              axis=1)
        ets = np.concatenate(
            [e_s[:, i * 128:(i + 1) * 128].T for i in range(NT)],
            axis=1)
        in_maps.append({
            "dxts": np.ascontiguousarray(dxts).astype(bf),
            "dyts": np.ascontiguousarray(dyts).astype(bf),
            "ets": np.ascontiguousarray(ets).astype(bf),
            "consts": consts,
        })
    return in_maps


def _ln_host(z, eps=1e-6):
    m = z.mean()
    s = z.std(ddof=1)
    return (z - m) / (s + eps)


def kernel(E, Dx, Dy, token_emb, tokens, _trace=False):
    from concourse.bass_utils import run_bass_kernel_spmd

    key = "nc"
    if key not in _cache:
        _cache[key] = _build()
    nc = _cache[key]

    in_maps = _host_prep(E, Dx, Dy, token_emb, tokens)
    res = run_bass_kernel_spmd(nc, in_maps, core_ids=list(range(N_CORES)),
                               trace=_trace)
    _cache["last_result"] = res

    r = res.results
    x_full = np.concatenate(
        [np.asarray(r[k]["out_x"], np.float32).T.ravel()
         for k in range(N_CORES)])
    # out_a holds (a - mean) * 2^-10 in column layout [128, 2]
    a_cent = np.asarray(r[0]["out_a"], np.float32).T.ravel() * 2.0 ** 10
    yfac = 1024.0 / (a_cent.std(ddof=1) + 1e-6)
    y_full = np.concatenate(
        [np.asarray(r[k]["out_yv"]).astype(np.float32)[:, 0:NT].T.ravel()
         * yfac for k in range(N_CORES)])
    vs_raw = np.zeros(256, np.float64)
    for k in range(N_CORES):
        vs_raw += np.asarray(r[k]["out_yv"]).astype(np.float32)[:, NT:].T.ravel()
    vs = _ln_host(vs_raw.astype(np.float32))
    rho = np.concatenate(
        [np.asarray(r[k]["out_rho"]).astype(np.float32)
         for k in range(N_CORES)], axis=1)
    return np.concatenate([x_full, y_full, vs, rho.ravel()]).astype(np.float32)
